# revision 9
# baseline (speedup 1.0000x reference)
"""Trainium2 Bass kernel for nn_Attention_60155311948227 (sparse_attention).

Sharding: data-parallel over batch B=8 across the 8 NeuronCores (1 sample per
core); the four FC weights are replicated (each core DMAs its own copy).

Per-core pipeline (all GEMMs bf16 with fp32 PSUM accumulation):
  XCT  = x_context^T            (on-chip PE transposes, bf16)
  A^T  = sum_{7x7}(x)           (DVE reduce; /49 folded into Q epilogue)
  Q^T  = relu(qW @ A/49 + b) -> BN        [d1(part), n]
  K^T  = relu(kW @ xc^T + b) -> BN        [d1(part), m]   (kept, bf16)
  kn2  = column sums of K^T**2 (ones-matmul)  -> rk = 1/||k_row||
  qn2  -> rq
  S    = (Q^T)^T @ K^T          [n, m], then *rq (row) *rk (col, DMA-bcast)
         + amask (host-built additive -50 mask, DMA-bcast), softmax(x100)
  P^T  via PE transpose
  V    = xc @ vW^T (+b, relu, BN) computed as V^T then PE-transposed to
         V_nat [m(part), d2]; L2-normalized per row (TT-reduce + sqrt/recip)
  WV^T = V_nat^T-contraction with P^T     [d2(part), n]
  F^T  = relu(fW @ WV + b) -> BN          [D(part), n]  fp32
  out  = x + F broadcast over the 7x7 window
"""

import sys

import numpy as np

try:
    import concourse.bacc as bacc
except ImportError:  # pragma: no cover
    sys.path.insert(0, "/opt/trn_rl_repo")
    import concourse.bacc as bacc

import ml_dtypes

import concourse.bass as bass
import concourse.tile as tile
from concourse import mybir
from concourse import bass_utils
from concourse.masks import make_identity

F32 = mybir.dt.float32
BF16 = mybir.dt.bfloat16
AF = mybir.ActivationFunctionType
ALU = mybir.AluOpType
AX = mybir.AxisListType

BN_EPS = 1e-5
NEG_MASK = -50.0
TEMP_INV = 100.0
NORM_EPS = 1e-24

# Problem dims (full-size). The builder is parameterized so tests can build
# scaled-down variants for CoreSim.
FULL = dict(B=8, n=64, m=2048, D0=1024, C0=2048, D1=2048, D2=2048, KK=49)

P = 128


def build_program(cfg=None, num_devices=8):
    """Emit the SPMD per-core Bass program. Returns the compiled Bacc."""
    cfg = dict(FULL if cfg is None else cfg)
    n, m, D0, C0, D1, D2, KK = (
        cfg["n"], cfg["m"], cfg["D0"], cfg["C0"], cfg["D1"], cfg["D2"], cfg["KK"]
    )
    nc_d0, nc_c0, nc_d1, nc_d2, nc_m = D0 // P, C0 // P, D1 // P, D2 // P, m // P
    n_nt = max(1, m // 512)          # 512-wide moving-dim tiles
    NT = m // n_nt                   # moving tile width
    inv_kk = 1.0 / KK

    nc = bacc.Bacc("TRN2", target_bir_lowering=False, debug=False,
                   num_devices=num_devices)

    def din(name, shape, dt=F32):
        return nc.dram_tensor(name, shape, dt, kind="ExternalInput").ap()

    x_in = din("x", [n, D0, KK])
    xc_in = din("xc", [m, C0])
    wqt = din("wqt", [D0, D1])
    wkt = din("wkt", [C0, D1])
    wvt = din("wvt", [C0, D2])
    wft = din("wft", [D2, D0])
    amask = din("amask", [m], BF16)
    qcb = din("qcb", [P, nc_d1]); qcg = din("qcg", [P, nc_d1]); qc2 = din("qc2", [P, nc_d1])
    kcb = din("kcb", [P, nc_d1]); kcg = din("kcg", [P, nc_d1]); kc2 = din("kc2", [P, nc_d1])
    vcb = din("vcb", [P, nc_d2]); vcg = din("vcg", [P, nc_d2]); vc2 = din("vc2", [P, nc_d2])
    fcb = din("fcb", [P, nc_d0]); fcg = din("fcg", [P, nc_d0]); fc2 = din("fc2", [P, nc_d0])
    out_d = nc.dram_tensor("out", [n, D0, KK], F32, kind="ExternalOutput").ap()

    with tile.TileContext(nc) as tc:
        with (
            tc.tile_pool(name="consts", bufs=1) as consts,
            tc.tile_pool(name="bigmat", bufs=1) as bigmat,
            tc.tile_pool(name="strips", bufs=3) as strips,
            tc.tile_pool(name="smalls", bufs=2) as smalls,
            tc.tile_pool(name="wides", bufs=1) as wides,
            tc.tile_pool(name="xpool", bufs=2) as xpool,
            tc.tile_pool(name="ps", bufs=1, space="PSUM") as ps,
            tc.tile_pool(name="dscr", bufs=1, space="DRAM") as dscr,
        ):
            # ---------------- constants ----------------
            ident = consts.tile([P, P], BF16)
            make_identity(nc, ident)
            ones_col = consts.tile([P, 1], BF16)
            nc.vector.memset(ones_col, 1.0)
            eps_col = consts.tile([P, 1], F32)
            nc.vector.memset(eps_col, NORM_EPS)

            def cload(ap_in, nch):
                t = consts.tile([P, nch], F32, name=f"c_{ap_in.tensor.name}")
                nc.sync.dma_start(out=t, in_=ap_in)
                return t

            qcb_t = cload(qcb, nc_d1); qcg_t = cload(qcg, nc_d1); qc2_t = cload(qc2, nc_d1)
            kcb_t = cload(kcb, nc_d1); kcg_t = cload(kcg, nc_d1); kc2_t = cload(kc2, nc_d1)
            vcb_t = cload(vcb, nc_d2); vcg_t = cload(vcg, nc_d2); vc2_t = cload(vc2, nc_d2)
            fcb_t = cload(fcb, nc_d0); fcg_t = cload(fcg, nc_d0); fc2_t = cload(fc2, nc_d0)

            amask_bc = consts.tile([n, m], BF16)
            nc.gpsimd.dma_start(
                out=amask_bc,
                in_=bass.AP(tensor=amask.tensor, offset=amask.offset,
                            ap=[[0, n]] + list(amask.ap)),
            )

            # ---------------- XCT: transpose x_context ----------------
            # XCT[p, c, i*P+w] = xc[i*P+w, c*P+p]  (bf16)
            xct = bigmat.tile([P, nc_c0, m], BF16, tag="xct")
            for i in range(nc_m):
                xcs = strips.tile([P, C0], BF16, tag="strip", name="xcs")
                nc.gpsimd.dma_start(out=xcs, in_=xc_in[i * P:(i + 1) * P, :])
                tp = ps.tile([P, nc_c0, P], BF16, tag="A", name="tp_xc")
                for c in range(nc_c0):
                    nc.tensor.transpose(tp[:, c, :], xcs[:, c * P:(c + 1) * P], ident)
                nc.vector.tensor_copy(out=xct[:, :, i * P:(i + 1) * P], in_=tp)

            # ---------------- pooling: A^T = sum_k x ----------------
            nh = n // 2
            at = consts.tile([P, nc_d0, n], BF16)
            for dd in range(nc_d0):
                for h in range(2):
                    xt = xpool.tile([P, nh, KK], F32, tag="x", name="xt")
                    nc.sync.dma_start(
                        out=xt,
                        in_=x_in[h * nh:(h + 1) * nh, dd * P:(dd + 1) * P, :]
                        .transpose([1, 0, 2]))
                    asum = smalls.tile([P, nh], F32, name="asum")
                    nc.vector.reduce_sum(asum, xt, axis=AX.X)
                    nc.vector.tensor_copy(out=at[:, dd, h * nh:(h + 1) * nh],
                                          in_=asum)

            # ---------------- Q^T projection ----------------
            qt = consts.tile([P, nc_d1, n], BF16)
            qn2 = ps.tile([1, n], F32, tag="B")
            for j in range(nc_d1):
                qws = strips.tile([P, nc_d0, P], BF16, tag="strip", name="qws")
                nc.gpsimd.dma_start(
                    out=qws,
                    in_=wqt[:, j * P:(j + 1) * P].rearrange("(c p) w -> p c w", p=P))
                qp = ps.tile([P, n], F32, tag="A", name="qp")
                for c in range(nc_d0):
                    nc.tensor.matmul(qp, qws[:, c, :], at[:, c, :],
                                     start=(c == 0), stop=(c == nc_d0 - 1))
                q1 = smalls.tile([P, n], BF16, name="q1")
                nc.scalar.activation(q1, qp, AF.Relu, bias=qcb_t[:, j:j + 1],
                                     scale=inv_kk)
                nc.vector.tensor_scalar(out=qt[:, j, :], in0=q1,
                                        scalar1=qcg_t[:, j:j + 1],
                                        scalar2=qc2_t[:, j:j + 1],
                                        op0=ALU.mult, op1=ALU.add)
                qsq = smalls.tile([P, n], BF16, name="qsq")
                nc.scalar.activation(qsq, qt[:, j, :], AF.Square)
                nc.tensor.matmul(qn2, ones_col, qsq,
                                 start=(j == 0), stop=(j == nc_d1 - 1))
            # rq = 1/sqrt(qn2) as a [n, 1] column
            rq_row = smalls.tile([1, n], F32, name="rq_row")
            nc.scalar.activation(rq_row, qn2, AF.Sqrt, bias=eps_col[:1, :])
            scr_q = dscr.tile([n], F32, name="scr_q")
            nc.gpsimd.dma_start(out=scr_q, in_=rq_row)
            rq_col = smalls.tile([n, 1], F32, name="rq_col")
            nc.gpsimd.dma_start(out=rq_col,
                              in_=bass.AP(tensor=scr_q.tensor, offset=scr_q.offset,
                                          ap=[[1, n], [1, 1]]))
            nc.vector.reciprocal(rq_col, rq_col)

            # ---------------- K^T projection (kept in SBUF) ----------------
            kt = bigmat.tile([P, nc_d1, m], BF16, tag="ktv", name="kt")
            kn2 = ps.tile([1, m], F32, tag="B")
            mh = m // 2
            for j in range(nc_d1):
                kws = strips.tile([P, nc_c0, P], BF16, tag="strip", name="kws")
                nc.gpsimd.dma_start(
                    out=kws,
                    in_=wkt[:, j * P:(j + 1) * P].rearrange("(c p) w -> p c w", p=P))
                kp = ps.tile([P, m], F32, tag="A", name="kp")
                for c in range(nc_c0):
                    for nt in range(n_nt):
                        nc.tensor.matmul(kp[:, nt * NT:(nt + 1) * NT],
                                         kws[:, c, :],
                                         xct[:, c, nt * NT:(nt + 1) * NT],
                                         start=(c == 0), stop=(c == nc_c0 - 1))
                # relu+bias: ACT takes one half, DVE the other (parallel drain)
                ktj = kt[:, j, :]
                nc.scalar.activation(ktj[:, :mh], kp[:, :mh], AF.Relu,
                                     bias=kcb_t[:, j:j + 1])
                nc.vector.tensor_scalar(out=ktj[:, mh:], in0=kp[:, mh:],
                                        scalar1=kcb_t[:, j:j + 1], scalar2=0.0,
                                        op0=ALU.add, op1=ALU.max)
                nc.vector.tensor_scalar(out=ktj, in0=ktj,
                                        scalar1=kcg_t[:, j:j + 1],
                                        scalar2=kc2_t[:, j:j + 1],
                                        op0=ALU.mult, op1=ALU.add)
                ksq = strips.tile([P, m], BF16, tag="strip", name="ksq")
                nc.scalar.activation(ksq, ktj, AF.Square)
                for nt in range(n_nt):
                    nc.tensor.matmul(kn2[:, nt * NT:(nt + 1) * NT], ones_col,
                                     ksq[:, nt * NT:(nt + 1) * NT],
                                     start=(j == 0), stop=(j == nc_d1 - 1))
            # rk chain: sqrt -> scatter to [P, m/P] -> reciprocal -> bcast [n, m]
            rk_row = smalls.tile([1, m], F32, name="rk_row")
            nc.scalar.activation(rk_row, kn2, AF.Sqrt, bias=eps_col[:1, :])
            scr_k = dscr.tile([m], F32, name="scr_k")
            nc.gpsimd.dma_start(out=scr_k, in_=rk_row)
            rk128 = smalls.tile([P, nc_m], F32, name="rk128")
            nc.gpsimd.dma_start(out=rk128,
                              in_=bass.AP(tensor=scr_k.tensor, offset=scr_k.offset,
                                          ap=[[1, P], [P, nc_m]]))
            nc.vector.reciprocal(rk128, rk128)
            scr_k2 = dscr.tile([m], F32, name="scr_k2")
            nc.gpsimd.dma_start(
                out=bass.AP(tensor=scr_k2.tensor, offset=scr_k2.offset,
                            ap=[[1, P], [P, nc_m]]),
                in_=rk128)
            rk_bc = wides.tile([n, m], F32, name="rk_bc")
            nc.gpsimd.dma_start(out=rk_bc,
                              in_=bass.AP(tensor=scr_k2.tensor, offset=scr_k2.offset,
                                          ap=[[0, n], [1, m]]))

            # ---------------- S = Q K^T, softmax ----------------
            sp = ps.tile([n, m], F32, tag="B", name="sp")
            for j in range(nc_d1):
                for nt in range(n_nt):
                    nc.tensor.matmul(sp[:, nt * NT:(nt + 1) * NT], qt[:, j, :],
                                     kt[:, j, nt * NT:(nt + 1) * NT],
                                     start=(j == 0), stop=(j == nc_d1 - 1))
            s_sb = wides.tile([n, m], F32, name="s_sb")
            nc.vector.tensor_scalar(out=s_sb, in0=sp, scalar1=rq_col, scalar2=None,
                                    op0=ALU.mult)
            nc.vector.tensor_mul(s_sb, s_sb, rk_bc)
            nc.vector.tensor_add(s_sb, s_sb, amask_bc)
            mxn = smalls.tile([n, 1], F32, name="mxn")
            nc.vector.tensor_reduce(mxn, s_sb, axis=AX.X, op=ALU.max, negate=True)
            ebias = smalls.tile([n, 1], F32, name="ebias")
            nc.vector.tensor_scalar_mul(ebias, mxn, TEMP_INV)
            p_t = wides.tile([n, m], BF16, name="p_t")
            pden = smalls.tile([n, 1], F32, name="pden")
            nc.scalar.activation(p_t, s_sb, AF.Exp, bias=ebias, scale=TEMP_INV,
                                 accum_out=pden)
            nc.vector.reciprocal(pden, pden)
            nc.vector.tensor_scalar_mul(p_t, p_t, pden)
            # P^T
            ptp = ps.tile([P, nc_m, n], BF16, tag="B", name="ptp")
            for i in range(nc_m):
                nc.tensor.transpose(ptp[:, i, :], p_t[:, i * P:(i + 1) * P],
                                    ident[:n, :n])
            pt_sb = consts.tile([P, nc_m, n], BF16)
            nc.vector.tensor_copy(out=pt_sb, in_=ptp)

            # ---------------- V: computed as V^T, transposed to V_nat ----------
            v_nat = bigmat.tile([P, nc_m, D2], BF16, tag="ktv", name="v_nat")
            for j in range(nc_d2):
                vws = strips.tile([P, nc_c0, P], BF16, tag="strip", name="vws")
                nc.gpsimd.dma_start(
                    out=vws,
                    in_=wvt[:, j * P:(j + 1) * P].rearrange("(c p) w -> p c w", p=P))
                vp = ps.tile([P, m], F32, tag="A", name="vp")
                for c in range(nc_c0):
                    for nt in range(n_nt):
                        nc.tensor.matmul(vp[:, nt * NT:(nt + 1) * NT],
                                         vws[:, c, :],
                                         xct[:, c, nt * NT:(nt + 1) * NT],
                                         start=(c == 0), stop=(c == nc_c0 - 1))
                vtj = strips.tile([P, m], BF16, tag="strip", name="vtj")
                nc.scalar.activation(vtj[:, :mh], vp[:, :mh], AF.Relu,
                                     bias=vcb_t[:, j:j + 1])
                nc.vector.tensor_scalar(out=vtj[:, mh:], in0=vp[:, mh:],
                                        scalar1=vcb_t[:, j:j + 1], scalar2=0.0,
                                        op0=ALU.add, op1=ALU.max)
                nc.vector.tensor_scalar(out=vtj, in0=vtj,
                                        scalar1=vcg_t[:, j:j + 1],
                                        scalar2=vc2_t[:, j:j + 1],
                                        op0=ALU.mult, op1=ALU.add)
                vtp = ps.tile([P, nc_m, P], BF16, tag="B", name="vtp")
                for i in range(nc_m):
                    nc.tensor.transpose(vtp[:, i, :], vtj[:, i * P:(i + 1) * P],
                                        ident)
                nc.vector.tensor_copy(out=v_nat[:, :, j * P:(j + 1) * P], in_=vtp)
            # L2 normalize V rows (per token); invalid rows get ~garbage but are
            # multiplied by exactly-zero attention weights.
            for i in range(nc_m):
                vsq = strips.tile([P, D2], BF16, tag="strip", name="vsq")
                vn2 = smalls.tile([P, 1], F32, name="vn2")
                nc.scalar.activation(vsq, v_nat[:, i, :], AF.Square, accum_out=vn2)
                rv = smalls.tile([P, 1], F32, name="rv")
                nc.scalar.activation(rv, vn2, AF.Sqrt, bias=eps_col)
                nc.vector.reciprocal(rv, rv)
                nc.vector.tensor_scalar_mul(v_nat[:, i, :], v_nat[:, i, :], rv)

            # ---------------- WV^T = sum_i V_nat_i^T P^T_i ----------------
            wvt_sb = consts.tile([P, nc_d2, n], BF16)
            for j in range(nc_d2):
                wvp = ps.tile([P, n], F32, tag="A", name="wvp")
                for i in range(nc_m):
                    nc.tensor.matmul(wvp, v_nat[:, i, j * P:(j + 1) * P],
                                     pt_sb[:, i, :],
                                     start=(i == 0), stop=(i == nc_m - 1))
                nc.vector.tensor_copy(out=wvt_sb[:, j, :], in_=wvp)

            # ---------------- F^T projection (fp32 out) ----------------
            ft = consts.tile([P, nc_d0, n], F32)
            for dd in range(nc_d0):
                fws = strips.tile([P, nc_d2, P], BF16, tag="strip", name="fws")
                nc.gpsimd.dma_start(
                    out=fws,
                    in_=wft[:, dd * P:(dd + 1) * P].rearrange("(c p) w -> p c w", p=P))
                fp = ps.tile([P, n], F32, tag="A", name="fp")
                for j in range(nc_d2):
                    nc.tensor.matmul(fp, fws[:, j, :], wvt_sb[:, j, :],
                                     start=(j == 0), stop=(j == nc_d2 - 1))
                f1 = smalls.tile([P, n], F32, name="f1")
                nc.scalar.activation(f1, fp, AF.Relu, bias=fcb_t[:, dd:dd + 1])
                nc.vector.tensor_scalar(out=ft[:, dd, :], in0=f1,
                                        scalar1=fcg_t[:, dd:dd + 1],
                                        scalar2=fc2_t[:, dd:dd + 1],
                                        op0=ALU.mult, op1=ALU.add)

            # ---------------- out = x + F (broadcast over 7x7) ----------------
            for dd in range(nc_d0):
                for h in range(2):
                    xo = xpool.tile([P, nh, KK], F32, tag="x", name="xo")
                    nc.sync.dma_start(
                        out=xo,
                        in_=x_in[h * nh:(h + 1) * nh, dd * P:(dd + 1) * P, :]
                        .transpose([1, 0, 2]))
                    nc.vector.tensor_add(
                        xo, xo,
                        ft[:, dd, h * nh:(h + 1) * nh].unsqueeze(2)
                        .broadcast_to([P, nh, KK]))
                    nc.sync.dma_start(
                        out=out_d[h * nh:(h + 1) * nh, dd * P:(dd + 1) * P, :]
                        .transpose([1, 0, 2]),
                        in_=xo)

    nc.compile()
    return nc


_CACHED = {}
# test-harness hook: extra kwargs for run_bass_kernel_spmd (e.g. trace=True)
_RUN_KWARGS = {}


def _get_program():
    if "nc" not in _CACHED:
        _CACHED["nc"] = build_program()
    return _CACHED["nc"]


def _bn_consts(b, gamma, beta, mean, var, nch):
    g = (gamma / np.sqrt(var + BN_EPS)).astype(np.float32)
    b2 = (beta - g * mean).astype(np.float32)
    def fold(v):
        return np.ascontiguousarray(v.reshape(nch, P).T)
    return fold(b.astype(np.float32)), fold(g), fold(b2)


def kernel(**inputs):
    cfg = FULL
    B, n, m = cfg["B"], cfg["n"], cfg["m"]
    D0, C0, D1, D2, KK = cfg["D0"], cfg["C0"], cfg["D1"], cfg["D2"], cfg["KK"]

    x = np.asarray(inputs["x"], dtype=np.float32).reshape(B, n, D0, KK)
    xc = np.asarray(inputs["x_context"], dtype=np.float32)
    nvalid = np.asarray(inputs["num_valid_context_items"]).reshape(B).astype(np.int64)

    wqt = np.ascontiguousarray(np.asarray(inputs["q_W"], np.float32).T)
    wkt = np.ascontiguousarray(np.asarray(inputs["k_W"], np.float32).T)
    wvt = np.ascontiguousarray(np.asarray(inputs["v_W"], np.float32).T)
    wft = np.ascontiguousarray(np.asarray(inputs["f_W"], np.float32).T)

    qcb, qcg, qc2 = _bn_consts(inputs["q_b"], inputs["q_gamma"], inputs["q_beta"],
                               inputs["q_mean"], inputs["q_var"], D1 // P)
    kcb, kcg, kc2 = _bn_consts(inputs["k_b"], inputs["k_gamma"], inputs["k_beta"],
                               inputs["k_mean"], inputs["k_var"], D1 // P)
    vcb, vcg, vc2 = _bn_consts(inputs["v_b"], inputs["v_gamma"], inputs["v_beta"],
                               inputs["v_mean"], inputs["v_var"], D2 // P)
    fcb, fcg, fc2 = _bn_consts(inputs["f_b"], inputs["f_gamma"], inputs["f_beta"],
                               inputs["f_mean"], inputs["f_var"], D0 // P)

    ar = np.arange(m)
    in_maps = []
    for b in range(B):
        am = np.where(ar < nvalid[b], 0.0, NEG_MASK).astype(ml_dtypes.bfloat16)
        in_maps.append({
            "x": np.ascontiguousarray(x[b]),
            "xc": np.ascontiguousarray(xc[b]),
            "wqt": wqt, "wkt": wkt, "wvt": wvt, "wft": wft,
            "amask": am,
            "qcb": qcb, "qcg": qcg, "qc2": qc2,
            "kcb": kcb, "kcg": kcg, "kc2": kc2,
            "vcb": vcb, "vcg": vcg, "vc2": vc2,
            "fcb": fcb, "fcg": fcg, "fc2": fc2,
        })

    nc = _get_program()
    res = bass_utils.run_bass_kernel_spmd(nc, in_maps, core_ids=list(range(B)),
                                          **_RUN_KWARGS)
    _CACHED["last_results"] = res
    out = np.stack([r["out"] for r in res.results], axis=0)
    return out.reshape(B, n, D0, 7, 7).astype(np.float32)
